# revision 1
# baseline (speedup 1.0000x reference)
"""Trainium2 Bass kernel for nn_HeatEquation1D.

The reference applies a fixed 62x62 Crank-Nicolson step matrix 100 times to
u0[:, 1:-1] via lax.scan, then zero-pads the boundary columns.  Algebraically
that whole scan is a single matmul:

    out = u0 @ W64,   W64[1:63, 1:63] = (step_matrix^100).T,  zero elsewhere

(the zero rows/cols of W64 implement both the dropped boundary inputs and the
zero Dirichlet boundary outputs).  W64 is computed on the host in float64.

Device kernel (per core, pure data parallel over 8 cores):
  - u shard (65536, 64) f32 is processed in 64 blocks of 1024 rows.
  - A block is DMA'd as one contiguous (128, 8, 64) tile X: partition p holds
    8 consecutive rows (2 KiB contiguous per partition -> efficient DMA).
  - For each 128-column chunk c (= 2 rows/partition), PE transpose:
      T1[:, c] = X[:, 2c:2c+2, :].T   (128, 128), into one PSUM bank tile.
  - One DVE copy PSUM -> SBUF (T1s).
  - matmul with the *transposed chunk as stationary* and a host-built
    BD = block_diag(W64, W64) (128x128) as the moving operand:
      Y[:, c] = T1s[:, c].T @ BD
    Because T1s chunk columns are (row-pair r', feature f) interleaved, BD's
    block-diagonal structure applies W64 to each row of the pair and the
    result lands *batch-major* -- no second transpose is needed; Y has the
    exact same (partition, row, feature) layout as X.
  - One DVE copy PSUM -> SBUF, then one contiguous 256 KiB DMA out.

Per-core traffic: 2 x 16.78 MB ~= 93 us at ~360 GB/s; PE/DVE work fits under
that, so the kernel is memory-bound as intended.
"""

import numpy as np

BATCH = 524288
NX = 64
N_INNER = NX - 2
NUM_STEPS = 100
N_CORES = 8
ROWS_PER_CORE = BATCH // N_CORES          # 65536
P = 128
ROWS_PER_PART = 8                          # rows per partition per block
ROWS_PER_BLOCK = P * ROWS_PER_PART         # 1024
N_BLOCKS = ROWS_PER_CORE // ROWS_PER_BLOCK  # 64
CHUNKS = (ROWS_PER_PART * NX) // P         # 4 chunks of 128 columns

# Set by callers that want a profile; results object stashed in LAST_RESULTS.
TRACE = False
LAST_RESULTS = None

_NC_CACHE = {}


def _build_nc(reps=1, dma_only=False):
    from concourse import bacc, mybir
    from concourse.tile import TileContext

    nc = bacc.Bacc("TRN2", target_bir_lowering=False, debug=False)
    f32 = mybir.dt.float32

    u = nc.dram_tensor("u", [ROWS_PER_CORE, NX], f32, kind="ExternalInput")
    bd_d = nc.dram_tensor("bd", [P, P], f32, kind="ExternalInput")
    id_d = nc.dram_tensor("ident", [P, P], f32, kind="ExternalInput")
    out = nc.dram_tensor("out", [ROWS_PER_CORE, NX], f32, kind="ExternalOutput")

    u_r = u.rearrange("(nb p r) f -> nb p r f", p=P, r=ROWS_PER_PART)
    out_r = out.rearrange("(nb p r) f -> nb p r f", p=P, r=ROWS_PER_PART)

    with TileContext(nc) as tc:
        with (
            tc.tile_pool(name="consts", bufs=1) as cpool,
            tc.tile_pool(name="xin", bufs=4) as xpool,
            tc.tile_pool(name="t1s", bufs=3) as tpool,
            tc.tile_pool(name="yout", bufs=4) as ypool,
            tc.tile_pool(name="ps_t", bufs=2, space="PSUM") as pst,
            tc.tile_pool(name="ps_y", bufs=2, space="PSUM") as psy,
        ):
            bd_s = cpool.tile([P, P], f32)
            id_s = cpool.tile([P, P], f32)
            nc.sync.dma_start(out=bd_s[:], in_=bd_d[:])
            nc.sync.dma_start(out=id_s[:], in_=id_d[:])

            for _rep in range(reps):
                for nb in range(N_BLOCKS):
                    x = xpool.tile([P, ROWS_PER_PART, NX], f32)
                    nc.sync.dma_start(out=x[:], in_=u_r[nb])

                    if dma_only:
                        nc.sync.dma_start(out=out_r[nb], in_=x[:])
                        continue

                    t1p = pst.tile([P, CHUNKS, P], f32)
                    for c in range(CHUNKS):
                        nc.tensor.transpose(
                            t1p[:, c], x[:, 2 * c : 2 * c + 2, :], id_s[:]
                        )
                    t1s = tpool.tile([P, CHUNKS, P], f32)
                    nc.vector.tensor_copy(out=t1s[:], in_=t1p[:])

                    yp = psy.tile([P, CHUNKS, P], f32)
                    for c in range(CHUNKS):
                        nc.tensor.matmul(
                            yp[:, c], t1s[:, c], bd_s[:], start=True, stop=True
                        )
                    ys = ypool.tile([P, ROWS_PER_PART, NX], f32)
                    nc.vector.tensor_copy(out=ys[:], in_=yp[:])
                    nc.sync.dma_start(out=out_r[nb], in_=ys[:])

    nc.compile()
    return nc


def _host_matrices(step_matrix):
    m = np.asarray(step_matrix, dtype=np.float64)
    w_inner = np.linalg.matrix_power(m, NUM_STEPS).T  # right-multiplier, f64
    w64 = np.zeros((NX, NX), dtype=np.float64)
    w64[1 : NX - 1, 1 : NX - 1] = w_inner
    bd = np.zeros((P, P), dtype=np.float64)
    bd[:NX, :NX] = w64
    bd[NX:, NX:] = w64
    return bd.astype(np.float32)


def kernel(u0, step_matrix):
    global LAST_RESULTS
    from concourse.bass_utils import run_bass_kernel_spmd

    u0 = np.ascontiguousarray(np.asarray(u0, dtype=np.float32))
    assert u0.shape == (BATCH, NX), u0.shape

    bd = _host_matrices(step_matrix)
    ident = np.eye(P, dtype=np.float32)

    if "nc" not in _NC_CACHE:
        _NC_CACHE["nc"] = _build_nc()
    nc = _NC_CACHE["nc"]

    shards = np.split(u0, N_CORES, axis=0)
    in_maps = [{"u": s, "bd": bd, "ident": ident} for s in shards]
    res = run_bass_kernel_spmd(
        nc, in_maps, core_ids=list(range(N_CORES)), trace=TRACE
    )
    LAST_RESULTS = res
    return np.concatenate([r["out"] for r in res.results], axis=0)



# revision 3
# speedup vs baseline: 1.1940x; 1.1940x over previous
"""Trainium2 Bass kernel for nn_HeatEquation1D.

The reference applies a fixed 62x62 Crank-Nicolson step matrix 100 times to
u0[:, 1:-1] via lax.scan, then zero-pads the boundary columns.  Algebraically
that whole scan is a single matmul:

    out = u0 @ W64,   W64[1:63, 1:63] = (step_matrix^100).T,  zero elsewhere

W64 is computed on the host in float64.  The matmul runs in bf16 (inputs are
rounded to bf16; accumulation is f32) -- rel err ~2e-3, well inside the 2e-2
gate -- which halves PE passes (no fp32 HI/LO split) and enables FWL.

Device kernel (per core, pure data parallel over 8 cores):
  - u shard (65536, 64) f32 is processed in 16 DMA blocks of 4096 rows (1 MiB
    DRAM-side).  Loads go through nc.gpsimd (SWDGE) which casts f32->bf16 in
    the DMA datapath, so SBUF holds bf16 and the load queue is distinct from
    the store queue (loads and stores overlap).
  - Each block is two half-blocks of 8 chunks; per 128-col chunk c:
      PE transpose:  t1p[:, c] = x_chunk.T        (bf16 in, f32 PSUM out)
      PE matmul:     yp[:, c]  = t1s[:, c].T @ BD (bf16 x bf16 -> f32 PSUM)
    with BD = block_diag(W64, W64) in bf16; the transposed chunk as stationary
    keeps the result batch-major (no second transpose).
  - DVE copies t1p (PSUM f32) -> t1s (SBUF bf16); ACT (scalar) copies yp
    (PSUM f32) -> ys (SBUF f32) -- the two copy streams run on different
    engines.
  - Stores are 1 MiB f32 DMAs issued on nc.sync (HWDGE ring).

Per-core HBM traffic: 2 x 16.78 MB ~= 94 us at 358 GB/s; PE (~40-55 us),
DVE (~35 us) and ACT (~30 us) all fit under that, so the kernel is
memory-bound as intended.
"""

import numpy as np
import ml_dtypes

BATCH = 524288
NX = 64
NUM_STEPS = 100
N_CORES = 8
ROWS_PER_CORE = BATCH // N_CORES          # 65536
P = 128

DMA_ROWS = 4096                            # rows per load/store DMA (1 MiB f32)
N_BLOCKS = ROWS_PER_CORE // DMA_ROWS       # 16
RPP = DMA_ROWS // P                        # 32 rows per partition per block
HALF = RPP // 2                            # 16 rows/partition per half-block
CHUNKS = (HALF * NX) // P                  # 8 transpose chunks per half-block

TRACE = False
LAST_RESULTS = None

_NC_CACHE = {}


def _build_nc():
    from concourse import bacc, mybir
    from concourse.tile import TileContext

    nc = bacc.Bacc("TRN2", target_bir_lowering=False, debug=False)
    f32 = mybir.dt.float32
    bf16 = mybir.dt.bfloat16

    u = nc.dram_tensor("u", [ROWS_PER_CORE, NX], f32, kind="ExternalInput")
    bd_d = nc.dram_tensor("bd", [P, P], bf16, kind="ExternalInput")
    id_d = nc.dram_tensor("ident", [P, P], bf16, kind="ExternalInput")
    out = nc.dram_tensor("out", [ROWS_PER_CORE, NX], f32, kind="ExternalOutput")

    u_r = u.rearrange("(nb p r) f -> nb p r f", p=P, r=RPP)
    out_r = out.rearrange("(nb p r) f -> nb p r f", p=P, r=RPP)

    with TileContext(nc) as tc:
        with (
            tc.tile_pool(name="consts", bufs=1) as cpool,
            tc.tile_pool(name="xin", bufs=3) as xpool,
            tc.tile_pool(name="t1s", bufs=4) as tpool,
            tc.tile_pool(name="yout", bufs=2) as ypool,
            tc.tile_pool(name="ps_t", bufs=2, space="PSUM") as pst,
            tc.tile_pool(name="ps_y", bufs=2, space="PSUM") as psy,
        ):
            bd_s = cpool.tile([P, P], bf16)
            id_s = cpool.tile([P, P], bf16)
            nc.sync.dma_start(out=bd_s[:], in_=bd_d[:])
            nc.sync.dma_start(out=id_s[:], in_=id_d[:])

            for nb in range(N_BLOCKS):
                x = xpool.tile([P, RPP, NX], bf16)
                nc.gpsimd.dma_start(out=x[:], in_=u_r[nb])  # casts f32->bf16

                ys = ypool.tile([P, 2, CHUNKS, P], f32)
                for h in range(2):
                    t1p = pst.tile([P, CHUNKS, P], bf16)
                    for c in range(CHUNKS):
                        r0 = h * HALF + 2 * c
                        nc.tensor.transpose(
                            t1p[:, c], x[:, r0 : r0 + 2, :], id_s[:]
                        )
                    t1s = tpool.tile([P, CHUNKS, P], bf16)
                    nc.vector.tensor_copy(out=t1s[:], in_=t1p[:])

                    yp = psy.tile([P, CHUNKS, P], f32)
                    for c in range(CHUNKS):
                        nc.tensor.matmul(
                            yp[:, c], t1s[:, c], bd_s[:], start=True, stop=True
                        )
                    nc.scalar.copy(out=ys[:, h], in_=yp[:])
                nc.sync.dma_start(out=out_r[nb], in_=ys[:])

    nc.compile()
    return nc


def _host_matrices(step_matrix):
    m = np.asarray(step_matrix, dtype=np.float64)
    w_inner = np.linalg.matrix_power(m, NUM_STEPS).T  # right-multiplier, f64
    w64 = np.zeros((NX, NX), dtype=np.float64)
    w64[1 : NX - 1, 1 : NX - 1] = w_inner
    bd = np.zeros((P, P), dtype=np.float64)
    bd[:NX, :NX] = w64
    bd[NX:, NX:] = w64
    return bd.astype(ml_dtypes.bfloat16)


def kernel(u0, step_matrix):
    global LAST_RESULTS
    from concourse.bass_utils import run_bass_kernel_spmd

    u0 = np.ascontiguousarray(np.asarray(u0, dtype=np.float32))
    assert u0.shape == (BATCH, NX), u0.shape

    bd = _host_matrices(step_matrix)
    ident = np.eye(P, dtype=np.float32).astype(ml_dtypes.bfloat16)

    if "nc" not in _NC_CACHE:
        _NC_CACHE["nc"] = _build_nc()
    nc = _NC_CACHE["nc"]

    shards = np.split(u0, N_CORES, axis=0)
    in_maps = [{"u": s, "bd": bd, "ident": ident} for s in shards]
    res = run_bass_kernel_spmd(
        nc, in_maps, core_ids=list(range(N_CORES)), trace=TRACE
    )
    LAST_RESULTS = res
    return np.concatenate([r["out"] for r in res.results], axis=0)
